# revision 7
# baseline (speedup 1.0000x reference)
"""Cosine multi-head attention (h=1) Trainium2 kernel.

Math (reference):
    context = query @ Wq.T + bq                  [B, S, HD]
    ctx     = context * weight_tensor[0]         (elementwise over HD)
    ctx_n   = ctx / max(||ctx||_2, eps)          (normalize over HD)
    scores  = ctx_n @ ctx_n.T                    [B, S, S]
    out     = softmax(scores, axis=-1)

Device strategy (8 cores, SPMD):
    core c handles batch b = c//2, row-half h = c%2.  The host rotates the
    batch's rows so each core's own 2048 rows come first, transposes to
    qT [D, S] (so the device never needs an on-chip transpose of q), and
    folds weight_tensor into Wq:  M = diag(w) @ Wq,  c0 = w * bq.

    On device:
      CT[hd, s] = sum_d M[hd, d] qT[d, s] + c0[hd]   (PSUM-resident, [120, 4096])
      norm2 broadcast over partitions via ones-matmul; inv = 1/max(sqrt, eps)
      Cn = CT * inv                                   [120, 4096]
      per 128-row chunk i of first 2048 rows:
         R = Cn[:, i-chunk].T @ Cn                    (PE, PSUM)
         E = exp(R) with fused row-sum (ACT accum_out)
         out_rows = E * (1/rowsum)                    (DVE) -> DMA out
    Softmax needs no max-subtraction: scores are cosines in [-1, 1].

    Output columns of h=1 cores are rotated by 2048; the host gather undoes it.
"""

import numpy as np
from contextlib import ExitStack

B, S, D, HD = 4, 4096, 1024, 120
ROWS = S // 2  # rows of the score matrix each core produces
EPS = 1e-12
N_CORES = 8

_NC_CACHE = {}


def _build_nc():
    import concourse.bacc as bacc
    import concourse.tile as tile
    from concourse import mybir

    f32 = mybir.dt.float32
    nc = bacc.Bacc("TRN2", target_bir_lowering=False, debug=False,
                   num_devices=N_CORES)

    qT = nc.declare_dram_parameter("qT", [D, S], f32, isOutput=False)
    mt = nc.declare_dram_parameter("mt", [D, HD], f32, isOutput=False)
    c0 = nc.declare_dram_parameter("c0", [1, HD], f32, isOutput=False)
    out = nc.declare_dram_parameter("out", [ROWS, S], f32, isOutput=True)

    DC = D // 128   # 8 contraction chunks
    KC = S // 512   # 8 column groups of 512

    with ExitStack() as ctx:
        tc = ctx.enter_context(tile.TileContext(nc))
        singles = ctx.enter_context(tc.tile_pool(name="singles", bufs=1))
        qpool = ctx.enter_context(tc.tile_pool(name="qpool", bufs=2))
        work = ctx.enter_context(tc.tile_pool(name="work", bufs=1))
        epool = ctx.enter_context(tc.tile_pool(name="epool", bufs=3))
        spool = ctx.enter_context(tc.tile_pool(name="spool", bufs=4))
        ps = ctx.enter_context(tc.tile_pool(name="ps", bufs=2, space="PSUM"))

        # --- constants ---
        mt_sb = singles.tile([128, DC, HD], f32, tag="mt")
        nc.sync.dma_start(out=mt_sb[:], in_=mt.rearrange("(c p) h -> p c h", p=128))
        c0_sb = singles.tile([1, HD], f32, tag="c0")
        nc.sync.dma_start(out=c0_sb[:], in_=c0[:])
        ones_row = singles.tile([1, 512], f32, tag="ones_row")
        nc.vector.memset(ones_row[:], 1.0)
        ones_sq = singles.tile([HD, HD], f32, tag="ones_sq")
        nc.vector.memset(ones_sq[:], 1.0)

        # --- phase 1: CT = M @ qT + c0, accumulated in PSUM [120, 4096] ---
        ct_ps = [ps.tile([HD, 2048], f32, tag="ps4", name=f"ct_ps{_h}") for _h in range(2)]
        for c in range(DC):
            qchunk = qpool.tile([128, S], f32, tag="qt")
            nc.sync.dma_start(out=qchunk[:], in_=qT[c * 128:(c + 1) * 128, :])
            for k in range(KC):
                nc.tensor.matmul(
                    ct_ps[k // 4][:, (k % 4) * 512:(k % 4 + 1) * 512],
                    lhsT=mt_sb[:, c, :],
                    rhs=qchunk[:, k * 512:(k + 1) * 512],
                    start=(c == 0), stop=False,
                )
        for k in range(KC):
            nc.tensor.matmul(
                ct_ps[k // 4][:, (k % 4) * 512:(k % 4 + 1) * 512],
                lhsT=c0_sb[:],
                rhs=ones_row[:],
                start=False, stop=True,
            )
        ct_sb = work.tile([HD, S], f32, tag="ct")
        for half in range(2):
            nc.vector.tensor_copy(ct_sb[:, half * 2048:(half + 1) * 2048],
                                  ct_ps[half][:])

        # --- phase 2: column norms, broadcast across partitions ---
        ctsq = work.tile([HD, S], f32, tag="ctsq")
        nc.vector.tensor_mul(ctsq[:], ct_sb[:], ct_sb[:])
        n_ps = [ps.tile([HD, 2048], f32, tag="ps4", name=f"n_ps{_h}") for _h in range(2)]
        for k in range(KC):
            nc.tensor.matmul(
                n_ps[k // 4][:, (k % 4) * 512:(k % 4 + 1) * 512],
                lhsT=ones_sq[:],
                rhs=ctsq[:, k * 512:(k + 1) * 512],
                start=True, stop=True,
            )
        nrm = work.tile([HD, S], f32, tag="nrm")
        for half in range(2):
            nc.scalar.sqrt(nrm[:, half * 2048:(half + 1) * 2048], n_ps[half][:])
        nc.vector.tensor_scalar_max(nrm[:], nrm[:], EPS)
        invn = work.tile([HD, S], f32, tag="invn")
        nc.vector.reciprocal(invn[:], nrm[:])
        cn = work.tile([HD, S], f32, tag="cn")
        nc.vector.tensor_mul(cn[:], ct_sb[:], invn[:])

        # --- phase 3: gram + softmax over 16 row chunks ---
        from concourse import mybir as mb
        for i in range(ROWS // 128):
            lhs = cn[:, i * 128:(i + 1) * 128]
            e_sb = epool.tile([128, S], f32, tag="e")
            sums = spool.tile([128, 2], f32, tag="sums")
            for jg in range(2):
                r_ps = ps.tile([128, 2048], f32, tag="ps4")
                for k in range(4):
                    nc.tensor.matmul(
                        r_ps[:, k * 512:(k + 1) * 512],
                        lhsT=lhs,
                        rhs=cn[:, jg * 2048 + k * 512: jg * 2048 + (k + 1) * 512],
                        start=True, stop=True,
                    )
                nc.scalar.activation(
                    out=e_sb[:, jg * 2048:(jg + 1) * 2048],
                    in_=r_ps[:],
                    func=mb.ActivationFunctionType.Exp,
                    accum_out=sums[:, jg:jg + 1],
                )
            tot = spool.tile([128, 1], f32, tag="tot")
            nc.vector.tensor_add(tot[:], sums[:, 0:1], sums[:, 1:2])
            rec = spool.tile([128, 1], f32, tag="rec")
            nc.vector.reciprocal(rec[:], tot[:])
            nc.vector.tensor_scalar_mul(e_sb[:], e_sb[:], rec[:])
            nc.sync.dma_start(out=out[i * 128:(i + 1) * 128, :], in_=e_sb[:])

    nc.compile()
    return nc


def _get_nc():
    if "nc" not in _NC_CACHE:
        _NC_CACHE["nc"] = _build_nc()
    return _NC_CACHE["nc"]


def _register_ntff_hook():
    """Register the axon NTFF profile hook that the agent image's antenv
    package lacks (see trn_boot.py) so trace=True yields exec_time_ns."""
    import sys
    import types
    try:
        import antenv.axon_hooks  # noqa: F401
        return True
    except ImportError:
        pass
    try:
        from trn_agent_boot.trn_boot import _ntff_profile_via_ctypes
        hook = _ntff_profile_via_ctypes("/opt/axon/libaxon_pjrt.so")
    except Exception:
        return False
    if hook is None:
        return False
    mod = types.ModuleType("antenv.axon_hooks")
    mod._hook = hook
    mod.get_axon_ntff_profile_hook = lambda: mod._hook
    mod.set_axon_ntff_profile_hook = lambda h: setattr(mod, "_hook", h)
    sys.modules["antenv.axon_hooks"] = mod
    import antenv
    antenv.axon_hooks = mod
    return True


def profile_once(inputs, trace_cores=None):
    """Re-run the kernel with NTFF profiling; returns max exec_time_ns."""
    import concourse.bass_utils as bu

    _register_ntff_hook()
    # avoid the cloud artifact upload inside the trace path
    bu.upload_artifacts = lambda tmpdir: tmpdir

    query = np.asarray(inputs["query"], dtype=np.float32)
    Wq = np.asarray(inputs["Wq"], dtype=np.float32)
    bq = np.asarray(inputs["bq"], dtype=np.float32)
    w = np.asarray(inputs["weight_tensor"], dtype=np.float32)
    w0 = w.reshape(-1)[:HD]
    mt_np = np.ascontiguousarray((w0[:, None] * Wq).T)
    c0_np = np.ascontiguousarray((w0 * bq)[None, :])
    in_maps = []
    for c in range(N_CORES):
        b, h = c // 2, c % 2
        qb = query[b]
        if h:
            qb = np.concatenate([qb[ROWS:], qb[:ROWS]], axis=0)
        in_maps.append({"qT": np.ascontiguousarray(qb.T), "mt": mt_np,
                        "c0": c0_np})
    nc = _get_nc()
    import tempfile
    tmpdir = tempfile.mkdtemp(prefix="ntff_")
    r = bu.run_bass_kernel_spmd(nc, in_maps, list(range(N_CORES)),
                                trace=True, trace_cores=trace_cores,
                                tmpdir=tmpdir)
    print(f"trace dir: {tmpdir}")
    if r.exec_time_ns is not None:
        print(f"mean exec: {r.mean_exec_time_ns} ns, "
              f"max core: {r.max_exec_time_core_id}")
    return r.exec_time_ns


def kernel(**inputs):
    from concourse.bass_utils import run_bass_kernel_spmd

    query = np.asarray(inputs["query"], dtype=np.float32)
    Wq = np.asarray(inputs["Wq"], dtype=np.float32)
    bq = np.asarray(inputs["bq"], dtype=np.float32)
    w = np.asarray(inputs["weight_tensor"], dtype=np.float32)

    w0 = w.reshape(-1)[:HD]
    mt_np = np.ascontiguousarray((w0[:, None] * Wq).T)          # [D, HD]
    c0_np = np.ascontiguousarray((w0 * bq)[None, :])            # [1, HD]

    in_maps = []
    for c in range(N_CORES):
        b, h = c // 2, c % 2
        qb = query[b]
        if h:
            qb = np.concatenate([qb[ROWS:], qb[:ROWS]], axis=0)
        in_maps.append({
            "qT": np.ascontiguousarray(qb.T),
            "mt": mt_np,
            "c0": c0_np,
        })

    nc = _get_nc()
    res = run_bass_kernel_spmd(nc, in_maps, list(range(N_CORES))).results

    full = np.empty((B, S, S), dtype=np.float32)
    for c in range(N_CORES):
        b, h = c // 2, c % 2
        r = res[c]["out"]
        if h == 0:
            full[b, :ROWS] = r
        else:
            full[b, ROWS:, ROWS:] = r[:, :ROWS]
            full[b, ROWS:, :ROWS] = r[:, ROWS:]
    return full
